# revision 10
# baseline (speedup 1.0000x reference)
"""MoE routing kernel for Trainium2, expert-parallel across 8 NeuronCores.

Sharding: core c owns experts [8c, 8c+8). The gate/top-k/dispatch-position
computation runs on host as part of the sharding step; each core receives its
experts' dispatched token rows (transposed, bf16), its expert weights, and a
slice of tokens for the (replicated-weight) shared expert. Device computes the
grouped SwiGLU expert GEMMs + shared expert. Host gathers per-slot outputs and
does the weighted combine (unshard).
"""

import os

import numpy as np
import ml_dtypes

import bass_rust
import concourse.bass as bass
import concourse.mybir as mybir
from concourse.tile import TileContext
from concourse.vector_clock import ScopedClock
from concourse.bass_utils import run_bass_kernel_spmd

B, T, C = 2, 2048, 2048
N = B * T
E, H, HS = 64, 256, 512
TOPK = 6
NCORES = 8
ELOC = E // NCORES  # 8 experts per core
NLOC = N // NCORES  # 512 tokens per core for the shared expert
BF16 = mybir.dt.bfloat16
F32 = mybir.dt.float32
P = 128

_BF16_NP = ml_dtypes.bfloat16


# --------------------------------------------------------------------------
# Tile tail-drain fix: this walrus build allows at most one semaphore wait per
# instruction (none on Drain). Tile's end-of-context drain carries the whole
# global clock; emit a chain of single-wait NOPs on SP instead.
# --------------------------------------------------------------------------
def _patched_drain_and_barrier(self, tick_clock, wait_clock):
    carrier = self.nc.sync.nop(nofuse=True, hint="tail_wait_0")
    wait_clock.add_sem_waits(carrier.ins, ScopedClock({None: tick_clock.global_clock}))
    si = carrier.ins.sync_info
    waits = list(si.on_wait) if si else []
    upds = list(si.on_update) if si else []
    carrier.ins.sync_info = bass_rust.SyncInfo(on_wait=waits[:1], on_update=upds)
    for i, w in enumerate(waits[1:]):
        n2 = self.nc.sync.nop(nofuse=True, hint=f"tail_wait_{i + 1}")
        n2.ins.sync_info = bass_rust.SyncInfo(on_wait=[w], on_update=[])

    self.nc.sync.drain()
    self.nc.all_engine_barrier()
    assert self.sems is not None
    popped = self.nc._tile_sem_poison_stack.pop()
    assert popped is self._sem_poison
    self.nc.clear_and_free_semaphores(list(self.sems.allocated().values()))
    self.nc.all_engine_barrier()


_orig_add_instruction = TileContext._add_instruction


def _patched_add_instruction(self, inst):
    si = getattr(inst, "sync_info", None)
    if si is not None and len(si.on_wait) > 1:
        waits = list(si.on_wait)
        for w in waits[:-1]:
            nop = mybir.InstNoOp(
                name=self.nc.get_next_instruction_name(), ins=[], outs=[])
            nop.engine = inst.engine
            nop.sync_info = bass_rust.SyncInfo(on_wait=[w], on_update=[])
            _orig_add_instruction(self, nop)
        inst.sync_info = bass_rust.SyncInfo(
            on_wait=[waits[-1]], on_update=list(si.on_update))
    _orig_add_instruction(self, inst)


def _install_drain_fix():
    if getattr(TileContext, "_drain_fix_installed", False):
        return
    TileContext._drain_and_barrier = _patched_drain_and_barrier
    TileContext._add_instruction = _patched_add_instruction
    TileContext._drain_fix_installed = True


# --------------------------------------------------------------------------
# Device kernel
# --------------------------------------------------------------------------
_BUILD_CACHE = {}


def _build(cap):
    """Build the per-core Bass program; cap = padded per-expert capacity."""
    _install_drain_fix()
    nc = bass.Bass()

    xdT = nc.declare_dram_parameter("xdT", [ELOC, C, cap], BF16, isOutput=False)
    wup = nc.declare_dram_parameter("wup", [ELOC, C, 2 * H], BF16, isOutput=False)
    wdn = nc.declare_dram_parameter("wdn", [ELOC, H, C], BF16, isOutput=False)
    xsT = nc.declare_dram_parameter("xsT", [C, NLOC], BF16, isOutput=False)
    wsu = nc.declare_dram_parameter("wsu", [C, 2 * HS], BF16, isOutput=False)
    wsd = nc.declare_dram_parameter("wsd", [HS, C], BF16, isOutput=False)
    yr = nc.declare_dram_parameter("yr", [ELOC * cap, C], BF16, isOutput=True)
    ysh = nc.declare_dram_parameter("ysh", [NLOC, C], BF16, isOutput=True)

    KC = C // P          # 16 contraction chunks over C
    MU = (2 * H) // P    # 4 output chunks of up-proj (2H = 512)
    KH = H // P          # 2 contraction chunks over H
    NCC = C // 512       # 4 output column chunks of down-proj
    SC = cap // P        # slot chunks per expert
    assert cap % P == 0

    with TileContext(nc) as tc:
        with (
            tc.tile_pool(name="wu_sb", bufs=24) as wu_pool,
            tc.tile_pool(name="xd_sb", bufs=24) as xd_pool,
            tc.tile_pool(name="wd_sb", bufs=4) as wd_pool,
            tc.tile_pool(name="h_sb", bufs=8) as h_pool,
            tc.tile_pool(name="sg_sb", bufs=4) as sg_pool,
            tc.tile_pool(name="o_sb", bufs=6) as o_pool,
            tc.tile_pool(name="sh_sb", bufs=KC) as sh_pool,
            tc.tile_pool(name="pu", bufs=6, space="PSUM") as pu_pool,
            tc.tile_pool(name="pd", bufs=2, space="PSUM") as pd_pool,
        ):
            # ---------------- routed experts ------------------------------
            for e in range(ELOC):
                # up-projection: psum[m] = [128 of 2H, cap slots]
                xd_tiles = []
                for k in range(KC):
                    t = xd_pool.tile([P, cap], BF16, tag="xd")
                    nc.gpsimd.dma_start(
                        out=t[:], in_=xdT[e, k * P:(k + 1) * P, :])
                    xd_tiles.append(t)
                wu_tiles = []
                for k in range(KC):
                    wt = wu_pool.tile([P, 2 * H], BF16, tag="wu")
                    nc.sync.dma_start(out=wt[:], in_=wup[e, k * P:(k + 1) * P, :])
                    wu_tiles.append(wt)
                up_tiles = []
                for m in range(MU):
                    pt = pu_pool.tile([P, cap], F32, space="PSUM", tag="pu")
                    for k in range(KC):
                        nc.tensor.matmul(
                            out=pt[:],
                            lhsT=wu_tiles[k][:, m * P:(m + 1) * P],
                            rhs=xd_tiles[k][:],
                            start=(k == 0), stop=(k == KC - 1))
                    up_tiles.append(pt)
                # g = chunks 0..1 (first 256 channels), v = chunks 2..3
                h_tiles = []
                for j in range(KH):
                    sg = sg_pool.tile([P, cap], F32, tag="sg2")
                    nc.scalar.activation(sg[:], up_tiles[j][:],
                                         mybir.ActivationFunctionType.Silu)
                    ht = h_pool.tile([P, cap], BF16, tag="h2")
                    nc.vector.tensor_mul(ht[:], sg[:], up_tiles[KH + j][:])
                    h_tiles.append(ht)
                # down-projection: lhsT = h slot-chunk, rhs = w_down columns
                wd_tiles = []
                for k in range(KH):
                    t = wd_pool.tile([P, C], BF16, tag="wd")
                    nc.gpsimd.dma_start(
                        out=t[:], in_=wdn[e, k * P:(k + 1) * P, :])
                    wd_tiles.append(t)
                for ms in range(SC):
                    for ncc in range(NCC):
                        pt = pd_pool.tile([P, 512], F32, space="PSUM", tag="pd")
                        for k in range(KH):
                            nc.tensor.matmul(
                                out=pt[:],
                                lhsT=h_tiles[k][:, ms * P:(ms + 1) * P],
                                rhs=wd_tiles[k][:, ncc * 512:(ncc + 1) * 512],
                                start=(k == 0), stop=(k == KH - 1))
                        ot = o_pool.tile([P, 512], BF16, tag="ord")
                        nc.vector.tensor_copy(out=ot[:], in_=pt[:])
                        row0 = e * cap + ms * P
                        nc.scalar.dma_start(
                            out=yr[row0:row0 + P, ncc * 512:(ncc + 1) * 512],
                            in_=ot[:])
            # ---------------- shared expert (512 local tokens) -------------
            xs_tiles = []
            for k in range(KC):
                t = sh_pool.tile([P, NLOC], BF16, tag="xs")
                nc.gpsimd.dma_start(out=t[:], in_=xsT[k * P:(k + 1) * P, :])
                xs_tiles.append(t)

            wsu_tiles = []
            for k in range(KC):
                t = sh_pool.tile([P, 2 * HS], BF16, tag="wsu")
                nc.sync.dma_start(out=t[:], in_=wsu[k * P:(k + 1) * P, :])
                wsu_tiles.append(t)

            hsh_tiles = []  # [HS part chunks (4), NLOC] bf16, h = silu(g_s)*y_s
            for half in range(2):  # process 2H_S=1024 in halves of 512 cols
                ps_tiles = []
                for m in range(4):
                    mm = half * 4 + m
                    pt = pu_pool.tile([P, NLOC], F32, space="PSUM", tag="pu")
                    for k in range(KC):
                        nc.tensor.matmul(
                            out=pt[:],
                            lhsT=wsu_tiles[k][:, mm * P:(mm + 1) * P],
                            rhs=xs_tiles[k][:],
                            start=(k == 0), stop=(k == KC - 1))
                    ps_tiles.append(pt)
                if half == 0:
                    # channels 0:512 = y_s (chunk order: y first); move out of
                    # PSUM so the second half can reuse the banks
                    y_s_tiles = []
                    for j in range(4):
                        yt = sg_pool.tile([P, NLOC], F32, tag="ys")
                        nc.vector.tensor_copy(out=yt[:], in_=ps_tiles[j][:])
                        y_s_tiles.append(yt)
                else:
                    # channels 512:1024 = g_s; h = silu(g_s) * y_s
                    for j in range(4):
                        sg = sg_pool.tile([P, NLOC], F32, tag="sg")
                        nc.scalar.activation(sg[:], ps_tiles[j][:],
                                             mybir.ActivationFunctionType.Silu)
                        ht = h_pool.tile([P, NLOC], BF16, tag="h")
                        nc.vector.tensor_mul(ht[:], sg[:], y_s_tiles[j][:])
                        hsh_tiles.append(ht)

            wsd_tiles = []
            for k in range(4):  # HS = 512 -> 4 chunks
                t = wd_pool.tile([P, C], BF16, tag="wsd")
                nc.scalar.dma_start(out=t[:], in_=wsd[k * P:(k + 1) * P, :])
                wsd_tiles.append(t)
            for mt in range(NLOC // P):  # 4 token chunks
                for ncc in range(NCC):
                    pt = pd_pool.tile([P, 512], F32, space="PSUM", tag="pd")
                    for k in range(4):
                        nc.tensor.matmul(
                            out=pt[:],
                            lhsT=hsh_tiles[k][:, mt * P:(mt + 1) * P],
                            rhs=wsd_tiles[k][:, ncc * 512:(ncc + 1) * 512],
                            start=(k == 0), stop=(k == 3))
                    ot = o_pool.tile([P, 512], BF16, tag="osh")
                    nc.vector.tensor_copy(out=ot[:], in_=pt[:])
                    nc.scalar.dma_start(
                        out=ysh[mt * P:(mt + 1) * P, ncc * 512:(ncc + 1) * 512],
                        in_=ot[:])

    return nc


# --------------------------------------------------------------------------
# Host wrapper
# --------------------------------------------------------------------------
def kernel(x, w_gate, w_shared_up, w_shared_down, w_up, w_down):
    x_flat = x.reshape(-1, C)

    # ---- gate: sigmoid scores, top-6, normalized weights (f64 for a stable
    # ordering; ties in the fp32 reference are measure-zero) ----
    logits = x_flat.astype(np.float64) @ w_gate.astype(np.float64)
    scores = 1.0 / (1.0 + np.exp(-logits))
    topk_idx = np.argsort(-scores, axis=-1, kind="stable")[:, :TOPK]
    w = np.take_along_axis(scores, topk_idx, axis=-1)
    w = w / w.sum(-1, keepdims=True)

    # ---- dispatch positions (stable within each expert, slot-major order) --
    flat_e = topk_idx.reshape(-1)
    order = np.argsort(flat_e, kind="stable")
    sorted_e = flat_e[order]
    group_start = np.searchsorted(sorted_e, np.arange(E))
    pos = np.empty(N * TOPK, dtype=np.int64)
    pos[order] = np.arange(N * TOPK) - group_start[sorted_e]
    counts = np.bincount(flat_e, minlength=E)

    cap = 512
    mx = int(counts.max())
    if mx > cap:
        cap = ((mx + P - 1) // P) * P

    # ---- build per-core inputs ----
    xT_bf = np.ascontiguousarray(x_flat.T).astype(_BF16_NP)  # [C, N]
    wup_bf = w_up.astype(_BF16_NP)
    wdn_bf = w_down.astype(_BF16_NP)
    wsu_bf = w_shared_up.astype(_BF16_NP)
    wsd_bf = w_shared_down.astype(_BF16_NP)

    token_of_slot = np.arange(N * TOPK) // TOPK
    in_maps = []
    expert_tokens = []
    for e in range(E):
        slots = order[group_start[e]: group_start[e] + counts[e]]
        expert_tokens.append(token_of_slot[slots])
    for c in range(NCORES):
        xdT = np.zeros((ELOC, C, cap), dtype=_BF16_NP)
        for j in range(ELOC):
            tok = expert_tokens[c * ELOC + j]
            xdT[j][:, : len(tok)] = xT_bf[:, tok]
        xsT = np.ascontiguousarray(xT_bf[:, c * NLOC:(c + 1) * NLOC])
        in_maps.append({
            "xdT": xdT,
            "wup": wup_bf[c * ELOC:(c + 1) * ELOC],
            "wdn": wdn_bf[c * ELOC:(c + 1) * ELOC],
            "xsT": xsT,
            "wsu": wsu_bf,
            "wsd": wsd_bf,
        })

    if cap not in _BUILD_CACHE:
        _BUILD_CACHE[cap] = _build(cap)
    nc = _BUILD_CACHE[cap]

    res = run_bass_kernel_spmd(nc, in_maps, list(range(NCORES)))
    if res.exec_time_ns is not None:
        print(f"HW exec time: {res.exec_time_ns} ns", flush=True)

    # ---- host combine (unshard): gather per-slot rows, weight, sum ----
    yr_all = np.concatenate(
        [r["yr"].reshape(ELOC, cap, C) for r in res.results], axis=0)
    y_ts = yr_all[flat_e, pos].astype(np.float32)          # [N*K, C]
    routed = (y_ts.reshape(N, TOPK, C)
              * w.reshape(N, TOPK, 1).astype(np.float32)).sum(axis=1)
    shared = np.concatenate([r["ysh"] for r in res.results], axis=0).astype(np.float32)
    return (shared + routed).reshape(B, T, C).astype(np.float32)


# revision 11
# speedup vs baseline: 1.0191x; 1.0191x over previous
"""MoE routing kernel for Trainium2, expert-parallel across 8 NeuronCores.

Sharding: core c owns experts [8c, 8c+8). The gate/top-k/dispatch-position
computation runs on host as part of the sharding step; each core receives its
experts' dispatched token rows (transposed, bf16), its expert weights, and a
slice of tokens for the (replicated-weight) shared expert. Device computes the
grouped SwiGLU expert GEMMs + shared expert. Host gathers per-slot outputs and
does the weighted combine (unshard).
"""

import os

import numpy as np
import ml_dtypes

import bass_rust
import concourse.bass as bass
import concourse.mybir as mybir
from concourse.tile import TileContext
from concourse.vector_clock import ScopedClock
from concourse.bass_utils import run_bass_kernel_spmd

B, T, C = 2, 2048, 2048
N = B * T
E, H, HS = 64, 256, 512
TOPK = 6
NCORES = 8
ELOC = E // NCORES  # 8 experts per core
NLOC = N // NCORES  # 512 tokens per core for the shared expert
BF16 = mybir.dt.bfloat16
F32 = mybir.dt.float32
P = 128

_BF16_NP = ml_dtypes.bfloat16


# --------------------------------------------------------------------------
# Tile tail-drain fix: this walrus build allows at most one semaphore wait per
# instruction (none on Drain). Tile's end-of-context drain carries the whole
# global clock; emit a chain of single-wait NOPs on SP instead.
# --------------------------------------------------------------------------
def _patched_drain_and_barrier(self, tick_clock, wait_clock):
    carrier = self.nc.sync.nop(nofuse=True, hint="tail_wait_0")
    wait_clock.add_sem_waits(carrier.ins, ScopedClock({None: tick_clock.global_clock}))
    si = carrier.ins.sync_info
    waits = list(si.on_wait) if si else []
    upds = list(si.on_update) if si else []
    carrier.ins.sync_info = bass_rust.SyncInfo(on_wait=waits[:1], on_update=upds)
    for i, w in enumerate(waits[1:]):
        n2 = self.nc.sync.nop(nofuse=True, hint=f"tail_wait_{i + 1}")
        n2.ins.sync_info = bass_rust.SyncInfo(on_wait=[w], on_update=[])

    self.nc.sync.drain()
    self.nc.all_engine_barrier()
    assert self.sems is not None
    popped = self.nc._tile_sem_poison_stack.pop()
    assert popped is self._sem_poison
    self.nc.clear_and_free_semaphores(list(self.sems.allocated().values()))
    self.nc.all_engine_barrier()


_orig_add_instruction = TileContext._add_instruction


def _patched_add_instruction(self, inst):
    si = getattr(inst, "sync_info", None)
    if si is not None and len(si.on_wait) > 1:
        waits = list(si.on_wait)
        for w in waits[:-1]:
            nop = mybir.InstNoOp(
                name=self.nc.get_next_instruction_name(), ins=[], outs=[])
            nop.engine = inst.engine
            nop.sync_info = bass_rust.SyncInfo(on_wait=[w], on_update=[])
            _orig_add_instruction(self, nop)
        inst.sync_info = bass_rust.SyncInfo(
            on_wait=[waits[-1]], on_update=list(si.on_update))
    _orig_add_instruction(self, inst)


def _install_drain_fix():
    if getattr(TileContext, "_drain_fix_installed", False):
        return
    TileContext._drain_and_barrier = _patched_drain_and_barrier
    TileContext._add_instruction = _patched_add_instruction
    TileContext._drain_fix_installed = True


# --------------------------------------------------------------------------
# Device kernel
# --------------------------------------------------------------------------
_BUILD_CACHE = {}


def _build(cap):
    """Build the per-core Bass program; cap = padded per-expert capacity."""
    _install_drain_fix()
    nc = bass.Bass()

    xdT = nc.declare_dram_parameter("xdT", [ELOC, C, cap], BF16, isOutput=False)
    wup = nc.declare_dram_parameter("wup", [ELOC, C, 2 * H], BF16, isOutput=False)
    wdn = nc.declare_dram_parameter("wdn", [ELOC, H, C], BF16, isOutput=False)
    xsT = nc.declare_dram_parameter("xsT", [C, NLOC], BF16, isOutput=False)
    wsu = nc.declare_dram_parameter("wsu", [C, 2 * HS], BF16, isOutput=False)
    wsd = nc.declare_dram_parameter("wsd", [HS, C], BF16, isOutput=False)
    yr = nc.declare_dram_parameter("yr", [ELOC * cap, C], BF16, isOutput=True)
    ysh = nc.declare_dram_parameter("ysh", [NLOC, C], BF16, isOutput=True)

    KC = C // P          # 16 contraction chunks over C
    MU = (2 * H) // P    # 4 output chunks of up-proj (2H = 512)
    KH = H // P          # 2 contraction chunks over H
    NCC = C // 512       # 4 output column chunks of down-proj
    SC = cap // P        # slot chunks per expert
    assert cap % P == 0

    with TileContext(nc) as tc:
        with (
            tc.tile_pool(name="wu_sb", bufs=24) as wu_pool,
            tc.tile_pool(name="xd_sb", bufs=24) as xd_pool,
            tc.tile_pool(name="wd_sb", bufs=4) as wd_pool,
            tc.tile_pool(name="h_sb", bufs=8) as h_pool,
            tc.tile_pool(name="sg_sb", bufs=4) as sg_pool,
            tc.tile_pool(name="o_sb", bufs=6) as o_pool,
            tc.tile_pool(name="sh_sb", bufs=KC) as sh_pool,
            tc.tile_pool(name="pu", bufs=6, space="PSUM") as pu_pool,
            tc.tile_pool(name="pd", bufs=2, space="PSUM") as pd_pool,
        ):
            # ---------------- shared expert (512 local tokens) -------------
            xs_tiles = []
            for k in range(KC):
                t = sh_pool.tile([P, NLOC], BF16, tag="xs")
                nc.sync.dma_start(out=t[:], in_=xsT[k * P:(k + 1) * P, :])
                xs_tiles.append(t)

            wsu_tiles = []
            for k in range(KC):
                t = sh_pool.tile([P, 2 * HS], BF16, tag="wsu")
                nc.sync.dma_start(out=t[:], in_=wsu[k * P:(k + 1) * P, :])
                wsu_tiles.append(t)

            hsh_tiles = []  # [HS part chunks (4), NLOC] bf16, h = silu(g_s)*y_s
            for half in range(2):  # process 2H_S=1024 in halves of 512 cols
                ps_tiles = []
                for m in range(4):
                    mm = half * 4 + m
                    pt = pu_pool.tile([P, NLOC], F32, space="PSUM", tag="pu")
                    for k in range(KC):
                        nc.tensor.matmul(
                            out=pt[:],
                            lhsT=wsu_tiles[k][:, mm * P:(mm + 1) * P],
                            rhs=xs_tiles[k][:],
                            start=(k == 0), stop=(k == KC - 1))
                    ps_tiles.append(pt)
                if half == 0:
                    # channels 0:512 = y_s (chunk order: y first); move out of
                    # PSUM so the second half can reuse the banks
                    y_s_tiles = []
                    for j in range(4):
                        yt = sg_pool.tile([P, NLOC], F32, tag="ys")
                        nc.vector.tensor_copy(out=yt[:], in_=ps_tiles[j][:])
                        y_s_tiles.append(yt)
                else:
                    # channels 512:1024 = g_s; h = silu(g_s) * y_s
                    for j in range(4):
                        sg = sg_pool.tile([P, NLOC], F32, tag="sg")
                        nc.scalar.activation(sg[:], ps_tiles[j][:],
                                             mybir.ActivationFunctionType.Silu)
                        ht = h_pool.tile([P, NLOC], BF16, tag="h")
                        nc.vector.tensor_mul(ht[:], sg[:], y_s_tiles[j][:])
                        hsh_tiles.append(ht)

            wsd_tiles = []
            for k in range(4):  # HS = 512 -> 4 chunks
                t = wd_pool.tile([P, C], BF16, tag="wsd")
                nc.sync.dma_start(out=t[:], in_=wsd[k * P:(k + 1) * P, :])
                wsd_tiles.append(t)
            for mt in range(NLOC // P):  # 4 token chunks
                for ncc in range(NCC):
                    pt = pd_pool.tile([P, 512], F32, space="PSUM", tag="pd")
                    for k in range(4):
                        nc.tensor.matmul(
                            out=pt[:],
                            lhsT=hsh_tiles[k][:, mt * P:(mt + 1) * P],
                            rhs=wsd_tiles[k][:, ncc * 512:(ncc + 1) * 512],
                            start=(k == 0), stop=(k == 3))
                    ot = o_pool.tile([P, 512], BF16, tag="osh")
                    nc.vector.tensor_copy(out=ot[:], in_=pt[:])
                    nc.scalar.dma_start(
                        out=ysh[mt * P:(mt + 1) * P, ncc * 512:(ncc + 1) * 512],
                        in_=ot[:])

            # ---------------- routed experts ------------------------------
            for e in range(ELOC):
                # up-projection: psum[m] = [128 of 2H, cap slots]
                xd_tiles = []
                for k in range(KC):
                    t = xd_pool.tile([P, cap], BF16, tag="xd")
                    nc.gpsimd.dma_start(
                        out=t[:], in_=xdT[e, k * P:(k + 1) * P, :])
                    xd_tiles.append(t)
                wu_tiles = []
                for k in range(KC):
                    wt = wu_pool.tile([P, 2 * H], BF16, tag="wu")
                    nc.sync.dma_start(out=wt[:], in_=wup[e, k * P:(k + 1) * P, :])
                    wu_tiles.append(wt)
                up_tiles = []
                for m in range(MU):
                    pt = pu_pool.tile([P, cap], F32, space="PSUM", tag="pu")
                    for k in range(KC):
                        nc.tensor.matmul(
                            out=pt[:],
                            lhsT=wu_tiles[k][:, m * P:(m + 1) * P],
                            rhs=xd_tiles[k][:],
                            start=(k == 0), stop=(k == KC - 1))
                    up_tiles.append(pt)
                # g = chunks 0..1 (first 256 channels), v = chunks 2..3
                h_tiles = []
                for j in range(KH):
                    sg = sg_pool.tile([P, cap], F32, tag="sg2")
                    nc.scalar.activation(sg[:], up_tiles[j][:],
                                         mybir.ActivationFunctionType.Silu)
                    ht = h_pool.tile([P, cap], BF16, tag="h2")
                    nc.vector.tensor_mul(ht[:], sg[:], up_tiles[KH + j][:])
                    h_tiles.append(ht)
                # down-projection: lhsT = h slot-chunk, rhs = w_down columns
                wd_tiles = []
                for k in range(KH):
                    t = wd_pool.tile([P, C], BF16, tag="wd")
                    nc.gpsimd.dma_start(
                        out=t[:], in_=wdn[e, k * P:(k + 1) * P, :])
                    wd_tiles.append(t)
                for ms in range(SC):
                    for ncc in range(NCC):
                        pt = pd_pool.tile([P, 512], F32, space="PSUM", tag="pd")
                        for k in range(KH):
                            nc.tensor.matmul(
                                out=pt[:],
                                lhsT=h_tiles[k][:, ms * P:(ms + 1) * P],
                                rhs=wd_tiles[k][:, ncc * 512:(ncc + 1) * 512],
                                start=(k == 0), stop=(k == KH - 1))
                        ot = o_pool.tile([P, 512], BF16, tag="ord")
                        nc.vector.tensor_copy(out=ot[:], in_=pt[:])
                        row0 = e * cap + ms * P
                        nc.scalar.dma_start(
                            out=yr[row0:row0 + P, ncc * 512:(ncc + 1) * 512],
                            in_=ot[:])
    return nc


# --------------------------------------------------------------------------
# Host wrapper
# --------------------------------------------------------------------------
def kernel(x, w_gate, w_shared_up, w_shared_down, w_up, w_down):
    x_flat = x.reshape(-1, C)

    # ---- gate: sigmoid scores, top-6, normalized weights (f64 for a stable
    # ordering; ties in the fp32 reference are measure-zero) ----
    logits = x_flat.astype(np.float64) @ w_gate.astype(np.float64)
    scores = 1.0 / (1.0 + np.exp(-logits))
    topk_idx = np.argsort(-scores, axis=-1, kind="stable")[:, :TOPK]
    w = np.take_along_axis(scores, topk_idx, axis=-1)
    w = w / w.sum(-1, keepdims=True)

    # ---- dispatch positions (stable within each expert, slot-major order) --
    flat_e = topk_idx.reshape(-1)
    order = np.argsort(flat_e, kind="stable")
    sorted_e = flat_e[order]
    group_start = np.searchsorted(sorted_e, np.arange(E))
    pos = np.empty(N * TOPK, dtype=np.int64)
    pos[order] = np.arange(N * TOPK) - group_start[sorted_e]
    counts = np.bincount(flat_e, minlength=E)

    cap = 512
    mx = int(counts.max())
    if mx > cap:
        cap = ((mx + P - 1) // P) * P

    # ---- build per-core inputs ----
    xT_bf = np.ascontiguousarray(x_flat.T).astype(_BF16_NP)  # [C, N]
    wup_bf = w_up.astype(_BF16_NP)
    wdn_bf = w_down.astype(_BF16_NP)
    wsu_bf = w_shared_up.astype(_BF16_NP)
    wsd_bf = w_shared_down.astype(_BF16_NP)

    token_of_slot = np.arange(N * TOPK) // TOPK
    in_maps = []
    expert_tokens = []
    for e in range(E):
        slots = order[group_start[e]: group_start[e] + counts[e]]
        expert_tokens.append(token_of_slot[slots])
    for c in range(NCORES):
        xdT = np.zeros((ELOC, C, cap), dtype=_BF16_NP)
        for j in range(ELOC):
            tok = expert_tokens[c * ELOC + j]
            xdT[j][:, : len(tok)] = xT_bf[:, tok]
        xsT = np.ascontiguousarray(xT_bf[:, c * NLOC:(c + 1) * NLOC])
        in_maps.append({
            "xdT": xdT,
            "wup": wup_bf[c * ELOC:(c + 1) * ELOC],
            "wdn": wdn_bf[c * ELOC:(c + 1) * ELOC],
            "xsT": xsT,
            "wsu": wsu_bf,
            "wsd": wsd_bf,
        })

    if cap not in _BUILD_CACHE:
        _BUILD_CACHE[cap] = _build(cap)
    nc = _BUILD_CACHE[cap]

    res = run_bass_kernel_spmd(nc, in_maps, list(range(NCORES)))
    if res.exec_time_ns is not None:
        print(f"HW exec time: {res.exec_time_ns} ns", flush=True)

    # ---- host combine (unshard): gather per-slot rows, weight, sum ----
    yr_all = np.concatenate(
        [r["yr"].reshape(ELOC, cap, C) for r in res.results], axis=0)
    y_ts = yr_all[flat_e, pos].astype(np.float32)          # [N*K, C]
    routed = (y_ts.reshape(N, TOPK, C)
              * w.reshape(N, TOPK, 1).astype(np.float32)).sum(axis=1)
    shared = np.concatenate([r["ysh"] for r in res.results], axis=0).astype(np.float32)
    return (shared + routed).reshape(B, T, C).astype(np.float32)
